# revision 3
# baseline (speedup 1.0000x reference)
import numpy as np
import sys

sys.path.insert(0, "/opt/trn_rl_repo")

import ml_dtypes

BF16 = ml_dtypes.bfloat16

B, S, DIM = 4, 2048, 2048
N_HEADS, N_KV_HEADS, HEAD_DIM = 16, 4, 128
G = N_HEADS // N_KV_HEADS  # 4 q heads per kv head
EPS = 1.1920928955078125e-07
SCALE = 1.0 / np.sqrt(HEAD_DIM)
TOK = 2 * S  # tokens per core group (a batch pair)
NT = TOK // 128  # 32 token tiles per group
NTB = S // 128  # 16 token tiles per batch
NTS = NT // 4  # 8 token tiles per core slice (1024 tokens)
GROUPS = [[0, 1, 2, 3], [4, 5, 6, 7]]

_CACHE = {}


def _build_graph(repeat=1):
    import concourse.bass as bass
    import concourse.mybir as mybir
    from concourse import bacc
    from concourse.tile import TileContext
    from concourse.masks import make_identity

    f32 = mybir.dt.float32
    bf16 = mybir.dt.bfloat16

    nc = bacc.Bacc()
    xTs_d = nc.declare_dram_parameter("xTs", [NT, 128, 16 * 128], bf16, isOutput=False)
    wqkv_d = nc.declare_dram_parameter("wqkvT", [16, 128, 768], bf16, isOutput=False)
    wo_d = nc.declare_dram_parameter("woT", [4, 128, DIM], bf16, isOutput=False)
    frq_d = nc.declare_dram_parameter("frq", [4, NTB, 128, 128], bf16, isOutput=False)
    vis_d = nc.declare_dram_parameter("vis", [2, S], f32, isOutput=False)
    out_d = nc.declare_dram_parameter("out", [TOK // 4, DIM], bf16, isOutput=True)

    # DRAM scratch, double-buffered across repeat iterations
    npar = min(2, repeat)
    part = [
        [nc.dram_tensor(f"part{b}_{pp}", [S, DIM], bf16) for b in range(2)]
        for pp in range(npar)
    ]
    rs = [
        [nc.dram_tensor(f"rs{b}_{pp}", [S // 4, DIM], bf16) for b in range(2)]
        for pp in range(npar)
    ]

    with TileContext(nc) as tc:
        with (
            tc.tile_pool(name="consts", bufs=1) as consts,
            tc.tile_pool(name="singles", bufs=1) as singles,
            tc.tile_pool(name="xin", bufs=6) as xin,
            tc.tile_pool(name="scr", bufs=2) as scr,
            tc.tile_pool(name="qkvp", bufs=2) as qkvp,
            tc.tile_pool(name="st", bufs=2) as stp,
            tc.tile_pool(name="ot", bufs=2) as otp,
            tc.tile_pool(name="psS", bufs=3, space="PSUM") as psS,
            tc.tile_pool(name="psO", bufs=2, space="PSUM") as psO,
            tc.tile_pool(name="psD", bufs=1, space="PSUM") as psD,
            tc.tile_pool(name="psY", bufs=1, space="PSUM") as psY,
            tc.tile_pool(name="psT", bufs=1, space="PSUM") as psT,
        ):
            ident = consts.tile([128, 128], bf16)
            make_identity(nc, ident)
            ones_sb = consts.tile([128, 128], bf16)
            nc.vector.memset(ones_sb, 1.0)
            eps_sb = consts.tile([128, 1], f32)
            nc.vector.memset(eps_sb, EPS)

            for it in range(repeat):
                pp = it % npar
                # ---- resident tiles ----
                wqkv_sb = singles.tile([128, 16, 768], bf16, tag="wqkv")
                nc.sync.dma_start(
                    out=wqkv_sb, in_=wqkv_d[:, :, :].rearrange("c p f -> p c f")
                )
                wo_sb = singles.tile([128, 4, DIM], bf16, tag="wo")
                nc.sync.dma_start(
                    out=wo_sb, in_=wo_d[:, :, :].rearrange("c p f -> p c f")
                )
                cosq_sb = singles.tile([128, NTB, 128], bf16, tag="cosq")
                sinq_sb = singles.tile([128, NTB, 128], bf16, tag="sinq")
                cosk_sb = singles.tile([128, NTB, 128], bf16, tag="cosk")
                sink_sb = singles.tile([128, NTB, 128], bf16, tag="sink")
                for j, sb in enumerate((cosq_sb, sinq_sb, cosk_sb, sink_sb)):
                    nc.sync.dma_start(
                        out=sb, in_=frq_d[j, :, :, :].rearrange("t p d -> p t d")
                    )
                vis_sb = singles.tile([128, NT], f32, tag="vis")
                nc.sync.dma_start(
                    out=vis_sb, in_=vis_d[:, :].rearrange("b (t p) -> p (b t)", p=128)
                )
                # exp-bias mask: 0 where key visible, -50 where invisible
                visbias = singles.tile([128, NT], f32, tag="visbias")
                nc.vector.tensor_scalar(
                    out=visbias,
                    in0=vis_sb,
                    scalar1=50.0,
                    scalar2=-50.0,
                    op0=mybir.AluOpType.mult,
                    op1=mybir.AluOpType.add,
                )
                # ---- per batch: stage 1 (qkv+norm+rope+transpose), then attention ----
                for b in range(2):
                  qT_sb = qkvp.tile([128, G, S], bf16, tag="qT")
                  kT_sb = qkvp.tile([128, S], bf16, tag="kT")
                  v_sb = qkvp.tile([128, NTB, 128], bf16, tag="v")
                  def _emit_transposes(qr_, kr_, t_):
                      for hh in range(4):
                          pt = psT.tile([128, 128], bf16, tag="t")
                          nc.tensor.matmul(
                              pt,
                              lhsT=qr_[:, hh * 128 : (hh + 1) * 128],
                              rhs=ident,
                              is_transpose=True,
                          )
                          nc.vector.tensor_copy(
                              qT_sb[:, hh, t_ * 128 : (t_ + 1) * 128], pt
                          )
                      pt = psT.tile([128, 128], bf16, tag="t")
                      nc.tensor.matmul(pt, lhsT=kr_, rhs=ident, is_transpose=True)
                      nc.vector.tensor_copy(kT_sb[:, t_ * 128 : (t_ + 1) * 128], pt)
                  prev = None
                  for t in range(NTB):
                    tt = b * NTB + t
                    xt = xin.tile([128, 16, 128], bf16)
                    nc.sync.dma_start(
                        out=xt,
                        in_=xTs_d[tt, :, :].rearrange("p (c k) -> p c k", c=16),
                    )
                    ps_q = psS.tile([128, 512], f32, tag="s")
                    ps_kv = psO.tile([128, 512], f32, tag="o")
                    for kc in range(16):
                        nc.tensor.matmul(
                            ps_q,
                            lhsT=xt[:, kc, :],
                            rhs=wqkv_sb[:, kc, 0:512],
                            start=(kc == 0),
                            stop=(kc == 15),
                        )
                        nc.tensor.matmul(
                            ps_kv[:, 0:256],
                            lhsT=xt[:, kc, :],
                            rhs=wqkv_sb[:, kc, 512:768],
                            start=(kc == 0),
                            stop=(kc == 15),
                        )
                    # rstd = (mean(x^2)+eps)^-1/2; Sqrt avoided via
                    # Ln/Exp so the act engine stays on the Exp table set
                    ss = scr.tile([128, 5], f32, tag="ss")
                    lnms = scr.tile([128, 5], f32, tag="lnms")
                    rstd = scr.tile([128, 5], f32, tag="rstd")
                    sqj = scr.tile([128, 128], f32, tag="sqj")
                    for hh in range(4):
                        nc.scalar.activation(
                            out=sqj,
                            in_=ps_q[:, hh * 128 : (hh + 1) * 128],
                            func=mybir.ActivationFunctionType.Square,
                            accum_out=ss[:, hh : hh + 1],
                        )
                    nc.scalar.activation(
                        out=sqj,
                        in_=ps_kv[:, 0:128],
                        func=mybir.ActivationFunctionType.Square,
                        accum_out=ss[:, 4:5],
                    )
                    nc.scalar.activation(
                        out=lnms,
                        in_=ss,
                        func=mybir.ActivationFunctionType.Ln,
                        bias=eps_sb[:, :],
                        scale=1.0 / 128.0,
                    )
                    nc.scalar.activation(
                        out=rstd,
                        in_=lnms,
                        func=mybir.ActivationFunctionType.Exp,
                        scale=-0.5,
                    )
                    qn = scr.tile([128, 512], bf16, tag="qn")
                    kn = scr.tile([128, 128], bf16, tag="kn")
                    for hh in range(4):
                        nc.vector.tensor_scalar_mul(
                            qn[:, hh * 128 : (hh + 1) * 128],
                            ps_q[:, hh * 128 : (hh + 1) * 128],
                            rstd[:, hh : hh + 1],
                        )
                    nc.vector.tensor_scalar_mul(kn, ps_kv[:, 0:128], rstd[:, 4:5])
                    nc.vector.tensor_copy(v_sb[:, t, :], ps_kv[:, 128:256])
                    # rotate-half pairs: rq[2i] = -qn[2i+1]; rq[2i+1] = qn[2i]
                    rq = scr.tile([128, 512], bf16, tag="rq")
                    qnv = qn.rearrange("p (x two) -> p x two", two=2)
                    rqv = rq.rearrange("p (x two) -> p x two", two=2)
                    nc.vector.tensor_scalar_mul(rqv[:, :, 0], qnv[:, :, 1], -1.0)
                    nc.vector.tensor_copy(rqv[:, :, 1], qnv[:, :, 0])
                    rk = scr.tile([128, 128], bf16, tag="rk")
                    knv = kn.rearrange("p (x two) -> p x two", two=2)
                    rkv = rk.rearrange("p (x two) -> p x two", two=2)
                    nc.vector.tensor_scalar_mul(rkv[:, :, 0], knv[:, :, 1], -1.0)
                    nc.vector.tensor_copy(rkv[:, :, 1], knv[:, :, 0])
                    qr = scr.tile([128, 512], bf16, tag="qr")
                    for hh in range(4):
                        sl = slice(hh * 128, (hh + 1) * 128)
                        nc.vector.tensor_mul(qr[:, sl], qn[:, sl], cosq_sb[:, t, :])
                        nc.vector.tensor_mul(rq[:, sl], rq[:, sl], sinq_sb[:, t, :])
                    nc.vector.tensor_add(qr, qr, rq)
                    kr = scr.tile([128, 128], bf16, tag="kr")
                    nc.vector.tensor_mul(kr, kn, cosk_sb[:, t, :])
                    nc.vector.tensor_mul(rk, rk, sink_sb[:, t, :])
                    nc.vector.tensor_add(kr, kr, rk)
                    if prev is not None:
                        _emit_transposes(*prev)
                    prev = (qr, kr, t)
                  _emit_transposes(*prev)

                  # ---- stage 2: attention for this batch ----
                  if True:
                    for qb in range(4):
                        qsl = slice(qb * 512, (qb + 1) * 512)
                        ot_sb = otp.tile([128, G, 512], bf16, tag="ot")
                        for hh in range(G):
                            st_t = stp.tile([128, NTB, 512], bf16, tag="st")
                            po = psO.tile([128, 512], f32, tag="o")
                            pd = psD.tile([128, 512], f32, tag="d")
                            # scores emitted 2 ahead of PV so the in-order
                            # PE never stalls on the exp between them
                            def _pv(kt):
                                nc.tensor.matmul(
                                    po,
                                    lhsT=v_sb[:, kt, :],
                                    rhs=st_t[:, kt, :],
                                    start=(kt == 0),
                                    stop=(kt == NTB - 1),
                                )
                            for kt in range(NTB):
                                psc = psS.tile([128, 512], f32, tag="s")
                                nc.tensor.matmul(
                                    psc,
                                    lhsT=kT_sb[:, kt * 128 : (kt + 1) * 128],
                                    rhs=qT_sb[:, hh, qsl],
                                    start=True,
                                    stop=True,
                                )
                                nc.scalar.activation(
                                    out=st_t[:, kt, :],
                                    in_=psc,
                                    func=mybir.ActivationFunctionType.Exp,
                                    bias=visbias[:, b * NTB + kt : b * NTB + kt + 1],
                                )
                                if kt >= 2:
                                    _pv(kt - 2)
                            _pv(NTB - 2)
                            _pv(NTB - 1)
                            for kt in range(NTB):
                                nc.tensor.matmul(
                                    pd,
                                    lhsT=ones_sb,
                                    rhs=st_t[:, kt, :],
                                    start=(kt == 0),
                                    stop=(kt == NTB - 1),
                                )
                            rden = scr.tile([128, 512], f32, tag="rden")
                            nc.vector.reciprocal(rden, pd)
                            nc.vector.tensor_mul(ot_sb[:, hh, :], po, rden)
                        # ---- output projection for this (b, qb) block ----
                        for sub in range(4):
                            tt = b * NTB + qb * 4 + sub
                            for ncho in range(4):
                                py = psY.tile([128, 512], f32, tag="y")
                                for hh in range(4):
                                    nc.tensor.matmul(
                                        py,
                                        lhsT=ot_sb[:, hh, sub * 128 : (sub + 1) * 128],
                                        rhs=wo_sb[:, hh, ncho * 512 : (ncho + 1) * 512],
                                        start=(hh == 0),
                                        stop=(hh == 3),
                                    )
                                y_sb = scr.tile([128, 512], bf16, tag="y")
                                nc.vector.tensor_scalar_mul(
                                    y_sb, py, vis_sb[:, tt : tt + 1]
                                )
                                idx = qb * 4 + sub
                                nc.sync.dma_start(
                                    out=part[pp][b][
                                        idx * 128 : (idx + 1) * 128,
                                        ncho * 512 : (ncho + 1) * 512,
                                    ],
                                    in_=y_sb,
                                )

                  # ---- stage 3: one ReduceScatter per batch ----
                  nc.gpsimd.collective_compute(
                      "ReduceScatter",
                      mybir.AluOpType.add,
                      replica_groups=GROUPS,
                      ins=[part[pp][b].ap().opt()],
                      outs=[rs[pp][b].ap().opt()],
                  )
                  q4 = S // 4
                  nc.sync.dma_start(
                      out=out_d[b * q4 : (b + 1) * q4, :],
                      in_=rs[pp][b][:, :],
                  )
    nc.finalize()
    return nc


def _prep_inputs(x, wqkv, wo, q_norm_w, k_norm_w, freqs_cos, freqs_sin, vis_mask):
    """Per-core inputs. Core c: kv head h=c%4, batch pair p=c//4, holds
    token-quarter c%4 of the pair's x (AllGathered on device)."""
    cos = np.asarray(freqs_cos, np.float32)[:, 0, :]  # [S,128]
    sin = np.asarray(freqs_sin, np.float32)[:, 0, :]
    qw = np.asarray(q_norm_w, np.float32)
    kw = np.asarray(k_norm_w, np.float32)

    def swap_pairs(w):
        v = w.reshape(-1, 2)
        return np.stack([v[:, 1], v[:, 0]], axis=1).reshape(-1)

    cosq = (cos * qw[None, :] * SCALE).astype(BF16)
    sinq = (sin * swap_pairs(qw)[None, :] * SCALE).astype(BF16)
    cosk = (cos * kw[None, :]).astype(BF16)
    sink = (sin * swap_pairs(kw)[None, :]).astype(BF16)
    # [4, NTB, 128, 128]: table j, token tile t, token-in-tile p, dim d
    frq = np.stack([cosq, sinq, cosk, sink]).reshape(4, NTB, 128, 128)
    frq = np.ascontiguousarray(frq)

    x = np.asarray(x, np.float32)
    wqkv = np.asarray(wqkv, np.float32)
    wo = np.asarray(wo, np.float32)
    visf = np.asarray(vis_mask).astype(np.float32)

    # per-pair transposed x tiles: [NT, 128 feat, 16*128 tok]
    xT_pairs = []
    vis_pairs = []
    for p in range(2):
        xpair = np.concatenate([x[2 * p], x[2 * p + 1]], axis=0)  # [4096, 2048]
        xT = np.ascontiguousarray(
            xpair.reshape(NT, 128, 16, 128).transpose(0, 3, 2, 1).reshape(NT, 128, 2048)
        ).astype(BF16)
        xT_pairs.append(xT)
        vis_pairs.append(np.ascontiguousarray(visf[2 * p : 2 * p + 2]))

    # per-head weight slices (full, pre-gathered)
    w_slices = []
    for h in range(4):
        wq = wqkv[512 * h : 512 * (h + 1)]  # [512, 2048]
        wk = wqkv[2048 + 128 * h : 2048 + 128 * (h + 1)]
        wv = wqkv[2560 + 128 * h : 2560 + 128 * (h + 1)]
        wslice = np.concatenate([wq, wk, wv], axis=0)  # [768, 2048]
        wqkvT = np.ascontiguousarray(wslice.T.reshape(16, 128, 768)).astype(BF16)
        woT = np.ascontiguousarray(
            wo[:, 512 * h : 512 * (h + 1)].T.reshape(4, 128, DIM)
        ).astype(BF16)
        w_slices.append((wqkvT, woT))

    in_maps = []
    for c in range(8):
        h = c % 4
        r = c % 4
        p = c // 4
        wqkvT, woT = w_slices[h]
        in_maps.append(
            {
                "xTs": xT_pairs[p],
                "wqkvT": wqkvT,
                "woT": woT,
                "frq": frq,
                "vis": vis_pairs[p],
            }
        )
    return in_maps


class _Results:
    def __init__(self, results):
        self.results = results
        self.exec_time_ns = None


def _get_runner(rep=1):
    """Cached jitted executor (bass2jax PJRT lowering, built once)."""
    if ("runner", rep) in _CACHE:
        return _CACHE[("runner", rep)]
    import jax
    import jax.numpy as jnp
    from jax.sharding import Mesh, PartitionSpec, NamedSharding
    from jax.experimental.shard_map import shard_map
    import concourse.mybir as mybir
    from concourse.bass2jax import (
        _bass_exec_p,
        partition_id_tensor,
        install_neuronx_cc_hook,
    )

    if ("nc", rep) not in _CACHE:
        _CACHE[("nc", rep)] = _build_graph(repeat=rep)
    nc = _CACHE[("nc", rep)]
    install_neuronx_cc_hook()
    n_cores = 8

    partition_name = nc.partition_id_tensor.name if nc.partition_id_tensor else None
    in_names, out_names, out_avals = [], [], []
    for alloc in nc.m.functions[0].allocations:
        if not isinstance(alloc, mybir.MemoryLocationSet):
            continue
        name = alloc.memorylocations[0].name
        if alloc.kind == "ExternalInput":
            if name != partition_name:
                in_names.append(name)
        elif alloc.kind == "ExternalOutput":
            out_names.append(name)
            shape = tuple(alloc.tensor_shape)
            dtype = mybir.dt.np(alloc.dtype)
            out_avals.append(jax.core.ShapedArray(shape, dtype))
    n_params = len(in_names)
    n_outs = len(out_avals)
    all_in_names = in_names + out_names
    if partition_name is not None:
        all_in_names = all_in_names + [partition_name]

    donate = tuple(range(n_params, n_params + n_outs))

    def _body(*args):
        operands = list(args)
        if partition_name is not None:
            operands.append(partition_id_tensor())
        outs = _bass_exec_p.bind(
            *operands,
            out_avals=tuple(out_avals),
            in_names=tuple(all_in_names),
            out_names=tuple(out_names),
            lowering_input_output_aliases=(),
            sim_require_finite=True,
            sim_require_nnan=True,
            nc=nc,
        )
        return tuple(outs)

    devices = jax.devices()[:n_cores]
    mesh = Mesh(np.asarray(devices), ("core",))
    in_specs = (PartitionSpec("core"),) * (n_params + n_outs)
    out_specs = (PartitionSpec("core"),) * n_outs
    sharded = jax.jit(
        shard_map(
            _body, mesh=mesh, in_specs=in_specs, out_specs=out_specs, check_rep=False
        ),
        donate_argnums=donate,
        keep_unused=True,
    )
    zero_shardings = tuple(
        NamedSharding(mesh, PartitionSpec("core")) for _ in range(n_outs)
    )

    def _zeros():
        return tuple(
            jnp.zeros((n_cores * a.shape[0], *a.shape[1:]), a.dtype) for a in out_avals
        )

    zeros_fn = jax.jit(_zeros, out_shardings=zero_shardings)

    def run(in_maps):
        per_core = [[np.asarray(m[name]) for name in in_names] for m in in_maps]
        concat_in = [
            np.concatenate([per_core[c][i] for c in range(n_cores)], axis=0)
            for i in range(n_params)
        ]
        zs = zeros_fn()
        out_arrs = sharded(*concat_in, *zs)
        results = [
            {
                name: np.asarray(out_arrs[i]).reshape(n_cores, *out_avals[i].shape)[c]
                for i, name in enumerate(out_names)
            }
            for c in range(n_cores)
        ]
        return _Results(results)

    _CACHE[("runner", rep)] = run
    return run


def run_hw(in_maps, trace=False, rep=1):
    return _get_runner(rep)(in_maps)


def kernel(x, wqkv, wo, q_norm_w, k_norm_w, freqs_cos, freqs_sin, vis_mask):
    in_maps = _prep_inputs(
        x, wqkv, wo, q_norm_w, k_norm_w, freqs_cos, freqs_sin, vis_mask
    )
    res = run_hw(in_maps)
    full = np.zeros((B, S, DIM), np.float32)
    Q = S // 4  # 512 tokens per (core, batch)
    for c in range(8):
        p, r = c // 4, c % 4
        o = np.asarray(res.results[c]["out"], np.float32)  # [1024, 2048]
        for b in range(2):
            full[2 * p + b, Q * r : Q * (r + 1)] = o[b * Q : (b + 1) * Q]
    return full


# revision 4
# speedup vs baseline: 1.0783x; 1.0783x over previous
import numpy as np
import sys

sys.path.insert(0, "/opt/trn_rl_repo")

import ml_dtypes

BF16 = ml_dtypes.bfloat16

B, S, DIM = 4, 2048, 2048
N_HEADS, N_KV_HEADS, HEAD_DIM = 16, 4, 128
G = N_HEADS // N_KV_HEADS  # 4 q heads per kv head
EPS = 1.1920928955078125e-07
SCALE = 1.0 / np.sqrt(HEAD_DIM)
TOK = 2 * S  # tokens per core group (a batch pair)
NT = TOK // 128  # 32 token tiles per group
NTB = S // 128  # 16 token tiles per batch
NTS = NT // 4  # 8 token tiles per core slice (1024 tokens)
GROUPS = [[0, 1, 2, 3], [4, 5, 6, 7]]

_CACHE = {}


def _build_graph(repeat=1):
    import concourse.bass as bass
    import concourse.mybir as mybir
    from concourse import bacc
    from concourse.tile import TileContext
    from concourse.masks import make_identity

    f32 = mybir.dt.float32
    bf16 = mybir.dt.bfloat16

    nc = bacc.Bacc()
    xTs_d = nc.declare_dram_parameter("xTs", [NT, 128, 16 * 128], bf16, isOutput=False)
    wqkv_d = nc.declare_dram_parameter("wqkvT", [16, 128, 768], bf16, isOutput=False)
    wo_d = nc.declare_dram_parameter("woT", [4, 128, DIM], bf16, isOutput=False)
    frq_d = nc.declare_dram_parameter("frq", [4, NTB, 128, 128], bf16, isOutput=False)
    vis_d = nc.declare_dram_parameter("vis", [2, S], f32, isOutput=False)
    out_d = nc.declare_dram_parameter("out", [TOK // 4, DIM], bf16, isOutput=True)

    # DRAM scratch, double-buffered across repeat iterations
    npar = min(2, repeat)
    part = [
        [nc.dram_tensor(f"part{b}_{pp}", [S, DIM], bf16) for b in range(2)]
        for pp in range(npar)
    ]
    rs = [
        [nc.dram_tensor(f"rs{b}_{pp}", [S // 4, DIM], bf16) for b in range(2)]
        for pp in range(npar)
    ]

    with TileContext(nc) as tc:
        with (
            tc.tile_pool(name="consts", bufs=1) as consts,
            tc.tile_pool(name="singles", bufs=1) as singles,
            tc.tile_pool(name="xin", bufs=6) as xin,
            tc.tile_pool(name="scr", bufs=2) as scr,
            tc.tile_pool(name="qkvp", bufs=2) as qkvp,
            tc.tile_pool(name="st", bufs=2) as stp,
            tc.tile_pool(name="ot", bufs=2) as otp,
            tc.tile_pool(name="psS", bufs=2, space="PSUM") as psS,
            tc.tile_pool(name="psO", bufs=2, space="PSUM") as psO,
            tc.tile_pool(name="psD", bufs=2, space="PSUM") as psD,
            tc.tile_pool(name="psY", bufs=1, space="PSUM") as psY,
            tc.tile_pool(name="psT", bufs=1, space="PSUM") as psT,
        ):
            ident = consts.tile([128, 128], bf16)
            make_identity(nc, ident)
            ones_sb = consts.tile([128, 128], bf16)
            nc.vector.memset(ones_sb, 1.0)
            eps_sb = consts.tile([128, 1], f32)
            nc.vector.memset(eps_sb, EPS)

            for it in range(repeat):
                pp = it % npar
                # ---- resident tiles ----
                wqkv_sb = singles.tile([128, 16, 768], bf16, tag="wqkv")
                nc.sync.dma_start(
                    out=wqkv_sb, in_=wqkv_d[:, :, :].rearrange("c p f -> p c f")
                )
                wo_sb = singles.tile([128, 4, DIM], bf16, tag="wo")
                nc.sync.dma_start(
                    out=wo_sb, in_=wo_d[:, :, :].rearrange("c p f -> p c f")
                )
                cosq_sb = singles.tile([128, NTB, 128], bf16, tag="cosq")
                sinq_sb = singles.tile([128, NTB, 128], bf16, tag="sinq")
                cosk_sb = singles.tile([128, NTB, 128], bf16, tag="cosk")
                sink_sb = singles.tile([128, NTB, 128], bf16, tag="sink")
                for j, sb in enumerate((cosq_sb, sinq_sb, cosk_sb, sink_sb)):
                    nc.sync.dma_start(
                        out=sb, in_=frq_d[j, :, :, :].rearrange("t p d -> p t d")
                    )
                vis_sb = singles.tile([128, NT], f32, tag="vis")
                nc.sync.dma_start(
                    out=vis_sb, in_=vis_d[:, :].rearrange("b (t p) -> p (b t)", p=128)
                )
                # exp-bias mask: 0 where key visible, -50 where invisible
                visbias = singles.tile([128, NT], f32, tag="visbias")
                nc.vector.tensor_scalar(
                    out=visbias,
                    in0=vis_sb,
                    scalar1=50.0,
                    scalar2=-50.0,
                    op0=mybir.AluOpType.mult,
                    op1=mybir.AluOpType.add,
                )
                # ---- per batch: stage 1 (qkv+norm+rope+transpose), then attention ----
                for b in range(2):
                  qT_sb = qkvp.tile([128, G, S], bf16, tag="qT")
                  kT_sb = qkvp.tile([128, S], bf16, tag="kT")
                  v_sb = qkvp.tile([128, NTB, 128], bf16, tag="v")
                  def _emit_transposes(qr_, kr_, t_):
                      for hh in range(4):
                          pt = psT.tile([128, 128], bf16, tag="t")
                          nc.tensor.matmul(
                              pt,
                              lhsT=qr_[:, hh * 128 : (hh + 1) * 128],
                              rhs=ident,
                              is_transpose=True,
                          )
                          nc.vector.tensor_copy(
                              qT_sb[:, hh, t_ * 128 : (t_ + 1) * 128], pt
                          )
                      pt = psT.tile([128, 128], bf16, tag="t")
                      nc.tensor.matmul(pt, lhsT=kr_, rhs=ident, is_transpose=True)
                      nc.vector.tensor_copy(kT_sb[:, t_ * 128 : (t_ + 1) * 128], pt)
                  prev = None
                  for t in range(NTB):
                    tt = b * NTB + t
                    xt = xin.tile([128, 16, 128], bf16)
                    nc.sync.dma_start(
                        out=xt,
                        in_=xTs_d[tt, :, :].rearrange("p (c k) -> p c k", c=16),
                    )
                    ps_q = psS.tile([128, 512], f32, tag="s")
                    ps_kv = psO.tile([128, 512], f32, tag="o")
                    for kc in range(16):
                        nc.tensor.matmul(
                            ps_q,
                            lhsT=xt[:, kc, :],
                            rhs=wqkv_sb[:, kc, 0:512],
                            start=(kc == 0),
                            stop=(kc == 15),
                        )
                        nc.tensor.matmul(
                            ps_kv[:, 0:256],
                            lhsT=xt[:, kc, :],
                            rhs=wqkv_sb[:, kc, 512:768],
                            start=(kc == 0),
                            stop=(kc == 15),
                        )
                    # rstd = (mean(x^2)+eps)^-1/2; Sqrt avoided via
                    # Ln/Exp so the act engine stays on the Exp table set
                    ss = scr.tile([128, 5], f32, tag="ss")
                    lnms = scr.tile([128, 5], f32, tag="lnms")
                    rstd = scr.tile([128, 5], f32, tag="rstd")
                    sqj = scr.tile([128, 128], f32, tag="sqj")
                    for hh in range(4):
                        nc.scalar.activation(
                            out=sqj,
                            in_=ps_q[:, hh * 128 : (hh + 1) * 128],
                            func=mybir.ActivationFunctionType.Square,
                            accum_out=ss[:, hh : hh + 1],
                        )
                    nc.scalar.activation(
                        out=sqj,
                        in_=ps_kv[:, 0:128],
                        func=mybir.ActivationFunctionType.Square,
                        accum_out=ss[:, 4:5],
                    )
                    nc.scalar.activation(
                        out=lnms,
                        in_=ss,
                        func=mybir.ActivationFunctionType.Ln,
                        bias=eps_sb[:, :],
                        scale=1.0 / 128.0,
                    )
                    nc.scalar.activation(
                        out=rstd,
                        in_=lnms,
                        func=mybir.ActivationFunctionType.Exp,
                        scale=-0.5,
                    )
                    qn = scr.tile([128, 512], bf16, tag="qn")
                    kn = scr.tile([128, 128], bf16, tag="kn")
                    for hh in range(4):
                        nc.vector.tensor_scalar_mul(
                            qn[:, hh * 128 : (hh + 1) * 128],
                            ps_q[:, hh * 128 : (hh + 1) * 128],
                            rstd[:, hh : hh + 1],
                        )
                    nc.vector.tensor_scalar_mul(kn, ps_kv[:, 0:128], rstd[:, 4:5])
                    nc.vector.tensor_copy(v_sb[:, t, :], ps_kv[:, 128:256])
                    # rotate-half pairs: rq[2i] = -qn[2i+1]; rq[2i+1] = qn[2i]
                    rq = scr.tile([128, 512], bf16, tag="rq")
                    qnv = qn.rearrange("p (x two) -> p x two", two=2)
                    rqv = rq.rearrange("p (x two) -> p x two", two=2)
                    nc.vector.tensor_scalar_mul(rqv[:, :, 0], qnv[:, :, 1], -1.0)
                    nc.vector.tensor_copy(rqv[:, :, 1], qnv[:, :, 0])
                    rk = scr.tile([128, 128], bf16, tag="rk")
                    knv = kn.rearrange("p (x two) -> p x two", two=2)
                    rkv = rk.rearrange("p (x two) -> p x two", two=2)
                    nc.vector.tensor_scalar_mul(rkv[:, :, 0], knv[:, :, 1], -1.0)
                    nc.vector.tensor_copy(rkv[:, :, 1], knv[:, :, 0])
                    qr = scr.tile([128, 512], bf16, tag="qr")
                    for hh in range(4):
                        sl = slice(hh * 128, (hh + 1) * 128)
                        nc.vector.tensor_mul(qr[:, sl], qn[:, sl], cosq_sb[:, t, :])
                        nc.vector.tensor_mul(rq[:, sl], rq[:, sl], sinq_sb[:, t, :])
                    nc.vector.tensor_add(qr, qr, rq)
                    kr = scr.tile([128, 128], bf16, tag="kr")
                    nc.vector.tensor_mul(kr, kn, cosk_sb[:, t, :])
                    nc.vector.tensor_mul(rk, rk, sink_sb[:, t, :])
                    nc.vector.tensor_add(kr, kr, rk)
                    if prev is not None:
                        _emit_transposes(*prev)
                    prev = (qr, kr, t)
                  _emit_transposes(*prev)

                  # ---- stage 2: attention for this batch ----
                  if True:
                    for qb in range(4):
                        qsl = slice(qb * 512, (qb + 1) * 512)
                        ot_sb = otp.tile([128, G, 512], bf16, tag="ot")
                        for hh in range(G):
                            st_t = stp.tile([128, NTB, 512], bf16, tag="st")
                            po = psO.tile([128, 512], f32, tag="o")
                            pd = psD.tile([128, 512], f32, tag="d")
                            # scores emitted 2 ahead of PV so the in-order
                            # PE never stalls on the exp between them
                            def _pv(kt):
                                nc.tensor.matmul(
                                    po,
                                    lhsT=v_sb[:, kt, :],
                                    rhs=st_t[:, kt, :],
                                    start=(kt == 0),
                                    stop=(kt == NTB - 1),
                                )
                            for kt in range(NTB):
                                psc = psS.tile([128, 512], f32, tag="s")
                                nc.tensor.matmul(
                                    psc,
                                    lhsT=kT_sb[:, kt * 128 : (kt + 1) * 128],
                                    rhs=qT_sb[:, hh, qsl],
                                    start=True,
                                    stop=True,
                                )
                                nc.scalar.activation(
                                    out=st_t[:, kt, :],
                                    in_=psc,
                                    func=mybir.ActivationFunctionType.Exp,
                                    bias=visbias[:, b * NTB + kt : b * NTB + kt + 1],
                                )
                                if kt >= 1:
                                    _pv(kt - 1)
                            _pv(NTB - 1)
                            for kt in range(NTB):
                                nc.tensor.matmul(
                                    pd,
                                    lhsT=ones_sb,
                                    rhs=st_t[:, kt, :],
                                    start=(kt == 0),
                                    stop=(kt == NTB - 1),
                                )
                            rden = scr.tile([128, 512], f32, tag="rden")
                            nc.vector.reciprocal(rden, pd)
                            nc.vector.tensor_mul(ot_sb[:, hh, :], po, rden)
                        # ---- output projection for this (b, qb) block ----
                        for sub in range(4):
                            tt = b * NTB + qb * 4 + sub
                            for ncho in range(4):
                                py = psY.tile([128, 512], f32, tag="y")
                                for hh in range(4):
                                    nc.tensor.matmul(
                                        py,
                                        lhsT=ot_sb[:, hh, sub * 128 : (sub + 1) * 128],
                                        rhs=wo_sb[:, hh, ncho * 512 : (ncho + 1) * 512],
                                        start=(hh == 0),
                                        stop=(hh == 3),
                                    )
                                y_sb = scr.tile([128, 512], bf16, tag="y")
                                nc.vector.tensor_scalar_mul(
                                    y_sb, py, vis_sb[:, tt : tt + 1]
                                )
                                idx = qb * 4 + sub
                                nc.sync.dma_start(
                                    out=part[pp][b][
                                        idx * 128 : (idx + 1) * 128,
                                        ncho * 512 : (ncho + 1) * 512,
                                    ],
                                    in_=y_sb,
                                )

                  # ---- stage 3: one ReduceScatter per batch ----
                  nc.gpsimd.collective_compute(
                      "ReduceScatter",
                      mybir.AluOpType.add,
                      replica_groups=GROUPS,
                      ins=[part[pp][b].ap().opt()],
                      outs=[rs[pp][b].ap().opt()],
                  )
                  q4 = S // 4
                  nc.sync.dma_start(
                      out=out_d[b * q4 : (b + 1) * q4, :],
                      in_=rs[pp][b][:, :],
                  )
    nc.finalize()
    return nc


def _prep_inputs(x, wqkv, wo, q_norm_w, k_norm_w, freqs_cos, freqs_sin, vis_mask):
    """Per-core inputs. Core c: kv head h=c%4, batch pair p=c//4, holds
    token-quarter c%4 of the pair's x (AllGathered on device)."""
    cos = np.asarray(freqs_cos, np.float32)[:, 0, :]  # [S,128]
    sin = np.asarray(freqs_sin, np.float32)[:, 0, :]
    qw = np.asarray(q_norm_w, np.float32)
    kw = np.asarray(k_norm_w, np.float32)

    def swap_pairs(w):
        v = w.reshape(-1, 2)
        return np.stack([v[:, 1], v[:, 0]], axis=1).reshape(-1)

    cosq = (cos * qw[None, :] * SCALE).astype(BF16)
    sinq = (sin * swap_pairs(qw)[None, :] * SCALE).astype(BF16)
    cosk = (cos * kw[None, :]).astype(BF16)
    sink = (sin * swap_pairs(kw)[None, :]).astype(BF16)
    # [4, NTB, 128, 128]: table j, token tile t, token-in-tile p, dim d
    frq = np.stack([cosq, sinq, cosk, sink]).reshape(4, NTB, 128, 128)
    frq = np.ascontiguousarray(frq)

    x = np.asarray(x, np.float32)
    wqkv = np.asarray(wqkv, np.float32)
    wo = np.asarray(wo, np.float32)
    visf = np.asarray(vis_mask).astype(np.float32)

    # per-pair transposed x tiles: [NT, 128 feat, 16*128 tok]
    xT_pairs = []
    vis_pairs = []
    for p in range(2):
        xpair = np.concatenate([x[2 * p], x[2 * p + 1]], axis=0)  # [4096, 2048]
        xT = np.ascontiguousarray(
            xpair.reshape(NT, 128, 16, 128).transpose(0, 3, 2, 1).reshape(NT, 128, 2048)
        ).astype(BF16)
        xT_pairs.append(xT)
        vis_pairs.append(np.ascontiguousarray(visf[2 * p : 2 * p + 2]))

    # per-head weight slices (full, pre-gathered)
    w_slices = []
    for h in range(4):
        wq = wqkv[512 * h : 512 * (h + 1)]  # [512, 2048]
        wk = wqkv[2048 + 128 * h : 2048 + 128 * (h + 1)]
        wv = wqkv[2560 + 128 * h : 2560 + 128 * (h + 1)]
        wslice = np.concatenate([wq, wk, wv], axis=0)  # [768, 2048]
        wqkvT = np.ascontiguousarray(wslice.T.reshape(16, 128, 768)).astype(BF16)
        woT = np.ascontiguousarray(
            wo[:, 512 * h : 512 * (h + 1)].T.reshape(4, 128, DIM)
        ).astype(BF16)
        w_slices.append((wqkvT, woT))

    in_maps = []
    for c in range(8):
        h = c % 4
        r = c % 4
        p = c // 4
        wqkvT, woT = w_slices[h]
        in_maps.append(
            {
                "xTs": xT_pairs[p],
                "wqkvT": wqkvT,
                "woT": woT,
                "frq": frq,
                "vis": vis_pairs[p],
            }
        )
    return in_maps


class _Results:
    def __init__(self, results):
        self.results = results
        self.exec_time_ns = None


def _get_runner(rep=1):
    """Cached jitted executor (bass2jax PJRT lowering, built once)."""
    if ("runner", rep) in _CACHE:
        return _CACHE[("runner", rep)]
    import jax
    import jax.numpy as jnp
    from jax.sharding import Mesh, PartitionSpec, NamedSharding
    from jax.experimental.shard_map import shard_map
    import concourse.mybir as mybir
    from concourse.bass2jax import (
        _bass_exec_p,
        partition_id_tensor,
        install_neuronx_cc_hook,
    )

    if ("nc", rep) not in _CACHE:
        _CACHE[("nc", rep)] = _build_graph(repeat=rep)
    nc = _CACHE[("nc", rep)]
    install_neuronx_cc_hook()
    n_cores = 8

    partition_name = nc.partition_id_tensor.name if nc.partition_id_tensor else None
    in_names, out_names, out_avals = [], [], []
    for alloc in nc.m.functions[0].allocations:
        if not isinstance(alloc, mybir.MemoryLocationSet):
            continue
        name = alloc.memorylocations[0].name
        if alloc.kind == "ExternalInput":
            if name != partition_name:
                in_names.append(name)
        elif alloc.kind == "ExternalOutput":
            out_names.append(name)
            shape = tuple(alloc.tensor_shape)
            dtype = mybir.dt.np(alloc.dtype)
            out_avals.append(jax.core.ShapedArray(shape, dtype))
    n_params = len(in_names)
    n_outs = len(out_avals)
    all_in_names = in_names + out_names
    if partition_name is not None:
        all_in_names = all_in_names + [partition_name]

    donate = tuple(range(n_params, n_params + n_outs))

    def _body(*args):
        operands = list(args)
        if partition_name is not None:
            operands.append(partition_id_tensor())
        outs = _bass_exec_p.bind(
            *operands,
            out_avals=tuple(out_avals),
            in_names=tuple(all_in_names),
            out_names=tuple(out_names),
            lowering_input_output_aliases=(),
            sim_require_finite=True,
            sim_require_nnan=True,
            nc=nc,
        )
        return tuple(outs)

    devices = jax.devices()[:n_cores]
    mesh = Mesh(np.asarray(devices), ("core",))
    in_specs = (PartitionSpec("core"),) * (n_params + n_outs)
    out_specs = (PartitionSpec("core"),) * n_outs
    sharded = jax.jit(
        shard_map(
            _body, mesh=mesh, in_specs=in_specs, out_specs=out_specs, check_rep=False
        ),
        donate_argnums=donate,
        keep_unused=True,
    )
    zero_shardings = tuple(
        NamedSharding(mesh, PartitionSpec("core")) for _ in range(n_outs)
    )

    def _zeros():
        return tuple(
            jnp.zeros((n_cores * a.shape[0], *a.shape[1:]), a.dtype) for a in out_avals
        )

    zeros_fn = jax.jit(_zeros, out_shardings=zero_shardings)

    def run(in_maps):
        per_core = [[np.asarray(m[name]) for name in in_names] for m in in_maps]
        concat_in = [
            np.concatenate([per_core[c][i] for c in range(n_cores)], axis=0)
            for i in range(n_params)
        ]
        zs = zeros_fn()
        out_arrs = sharded(*concat_in, *zs)
        results = [
            {
                name: np.asarray(out_arrs[i]).reshape(n_cores, *out_avals[i].shape)[c]
                for i, name in enumerate(out_names)
            }
            for c in range(n_cores)
        ]
        return _Results(results)

    _CACHE[("runner", rep)] = run
    return run


def run_hw(in_maps, trace=False, rep=1):
    return _get_runner(rep)(in_maps)


def kernel(x, wqkv, wo, q_norm_w, k_norm_w, freqs_cos, freqs_sin, vis_mask):
    in_maps = _prep_inputs(
        x, wqkv, wo, q_norm_w, k_norm_w, freqs_cos, freqs_sin, vis_mask
    )
    res = run_hw(in_maps)
    full = np.zeros((B, S, DIM), np.float32)
    Q = S // 4  # 512 tokens per (core, batch)
    for c in range(8):
        p, r = c // 4, c % 4
        o = np.asarray(res.results[c]["out"], np.float32)  # [1024, 2048]
        for b in range(2):
            full[2 * p + b, Q * r : Q * (r + 1)] = o[b * Q : (b + 1) * Q]
    return full
